# revision 1
# baseline (speedup 1.0000x reference)
"""DropBlock kernel for Trainium2, 8 NeuronCores, batch-sharded data parallel.

Reference computation (B,C,H,W = 128,64,56,56, block=5, gamma=0.02):
    mask    = (noise < gamma)                       # (B,C,52,52) corner drops
    dilated = maxpool5x5_full_pad(mask)             # (B,C,56,56)
    block_mask = 1 - dilated
    out = block_mask * x * (numel / sum(block_mask))

Kernel formulation (exact, sign-mask):
    q = Sign(noise - gamma_lo) in {-1,0,+1}   # one ACT op per tile; the
        # subtract rides the activation bias and is computed in f32 inside
        # the ACT datapath, so the compare against 0.02f is exact
    dm = min over the 5x5 window of q       # -1 iff any corner drops
    block_mask = (dm > 0);  count = (N + sum(Sign(dm)))/2  (exact-zero dm
        # are measure-zero and shift the ~15M count by <1)
    scale = M/count = 2M/(M + S_global);  S summed by AllReduce.

Each core: 16 batches x 64 ch = 1024 images -> 8 tiles of 128 images
(images on partitions, image pixels along the free dimension).  The
5-wide separable window-min uses log-step shifts (3 tensor_tensor ops per
axis) on 1.0-padded buffers so no boundary special cases are needed; pad-
constant output columns of the horizontal chain are pre-set once.

Engine split (per tile): ACT produces q (one Sign), casts x to bf16, and
accumulates the Sign-sum of the pooled mask; DVE runs only the bf16
window-min tensor_tensor chain (2x mode) — keeping ACT under the DVE
cadence lets the tile scheduler dovetail adjacent tiles' chains into the
dependent-op semaphore gaps.  Tile 7's count (the only one on the
critical path) is split: ACT signs one half while DVE add-reduces the
other.  All compute stays off GpSimd — its TensorTensor is software-
emulated on real silicon and an order of magnitude slower than the cost
model's estimate.  The post-collective tail is per tile one 4x-mode
(dm>0)*scale tensor_scalar plus one 2x bf16 multiply into the cast x,
stored as bf16 (two roundings, ~0.2% rms; widened to f32 on the host).
The cross-partition count reduction+broadcast is one PE matmul against a
ones matrix.
"""

import sys

sys.path.insert(0, "/opt/trn_rl_repo")

import numpy as np

import concourse.bacc as bacc
import concourse.bass as bass
import concourse.tile as tile
import concourse.mybir as mybir
from concourse.bass_utils import run_bass_kernel_spmd

N_CORES = 8
B, C, H, W = 128, 64, 56, 56
BLK = 5
GAMMA = 0.02
NH, NW = H - (BLK - 1), W - (BLK - 1)  # 52, 52 noise dims
B_SH = B // N_CORES  # 16 batches per core
IMGS = B_SH * C  # 1024 images per core
P = 128  # partitions
NTILES = IMGS // P  # 8 tiles per core
NPIX = NH * NW  # 2704 noise pixels/image
OPIX = H * W  # 3136 out pixels/image
TROWS = NH + 2 * (BLK - 1)  # 60 rows in padded vertical buffer
TFLAT = TROWS * NW  # 3120
VPW = NW + 2 * (BLK - 1)  # 60 cols in padded horizontal buffer (4+52+4)
COUNT_M = float(B * C * H * W)  # 25690112.0

# Largest f32 strictly below 0.02f: noise > gamma_lo  <=>  noise >= 0.02f.
GAMMA_LO = float(np.nextafter(np.float32(GAMMA), np.float32(0)))

F32 = mybir.dt.float32
BF16 = mybir.dt.bfloat16
MIN = mybir.AluOpType.min
MULT = mybir.AluOpType.mult
ADD = mybir.AluOpType.add
IDENT = mybir.ActivationFunctionType.Identity
SIGN = mybir.ActivationFunctionType.Sign
RELU = mybir.ActivationFunctionType.Relu

X_BUFS = 2  # f32 x staging ring; each tile is cast to bf16 right away

_CACHE = {}


def _build(single_core=False, repeat=1, no_cc=False):
    """Build + compile the SPMD bass module once.

    single_core=True builds a collective-free variant (the per-core count is
    used directly as the global count) for cost-model simulation only.
    repeat>1 unrolls the whole pipeline k times (benchmarking only).
    no_cc=True skips the AllReduce on the 8-core build (timing probe only —
    results are wrong by the per-core/global count ratio).
    """
    nc = bacc.Bacc("TRN2", target_bir_lowering=False, debug=False,
                   num_devices=1 if single_core else N_CORES)
    noise_ap = nc.dram_tensor("noise", [IMGS, NPIX], F32,
                              kind="ExternalInput").ap()
    x_ap = nc.dram_tensor("x", [IMGS, OPIX], F32, kind="ExternalInput").ap()
    out_ap = nc.dram_tensor("out", [IMGS, OPIX], BF16,
                            kind="ExternalOutput").ap()

    with tile.TileContext(nc) as tc:
        with (
            tc.tile_pool(name="buf", bufs=1) as bp,
            tc.tile_pool(name="stats", bufs=1) as stats_pool,
            tc.tile_pool(name="ps", bufs=1, space="PSUM") as psum_pool,
            tc.tile_pool(name="dram", bufs=1, space="DRAM") as dram_pool,
        ):
            bufs = {
                "nraw": [bp.tile([P, NPIX], F32, name=f"nraw{s}")
                         for s in range(2)],
                "tb": [bp.tile([P, TFLAT], BF16, name=f"tb{s}")
                       for s in range(2)],
                "a": [bp.tile([P, (TROWS - 1) * NW], BF16, name="a0")],
                "bt": [bp.tile([P, (TROWS - 3) * NW], BF16, name="bt0")],
                "vp": [bp.tile([P, H * VPW], BF16, name="vp0")],
                "a2": [bp.tile([P, H * VPW], BF16, name="a20")],
                "b2": [bp.tile([P, H * VPW], BF16, name="b20")],
                "junk": [bp.tile([P, OPIX], BF16, name="junk0")],
                "x": [bp.tile([P, OPIX], F32, name=f"x{s}")
                      for s in range(X_BUFS)],
            }
            dmstore = bp.tile([P, NTILES * OPIX], BF16, name="dmstore")
            xbstore = bp.tile([P, NTILES * OPIX], BF16, name="xbstore")
            ones_sb = bp.tile([P, P], F32, name="ones")
            nc.gpsimd.memset(ones_sb[:], 1.0)
            partials = stats_pool.tile([P, NTILES], F32)
            gbias = stats_pool.tile([P, 1], F32)
            nc.vector.memset(gbias[:], -GAMMA_LO)
            # warm the ACT function tables on a 1-element tile during the
            # DMA lead-in; otherwise LoadActFuncSet (~1.3us) lands in front
            # of the first real op and stalls the pipeline.
            warm = stats_pool.tile([P, 1], F32)
            nc.scalar.activation(warm[:], gbias[:], IDENT, bias=gbias[:, 0:1])
            nc.scalar.activation(warm[:], warm[:], SIGN)
            nc.scalar.activation(warm[:], warm[:], RELU)

            # 1.0-pads written once; every later op touches only data rows.
            for tb in bufs["tb"]:
                nc.gpsimd.memset(tb[:, 0:(BLK - 1) * NW], 1.0)
                nc.gpsimd.memset(tb[:, (NH + BLK - 1) * NW:TFLAT], 1.0)
            a0 = bufs["a"][0]
            nc.gpsimd.memset(a0[:, 0:3 * NW], 1.0)
            nc.gpsimd.memset(a0[:, 56 * NW:(TROWS - 1) * NW], 1.0)
            bt0 = bufs["bt"][0]
            nc.gpsimd.memset(bt0[:, 0:NW], 1.0)
            # row 56 = product of pad rows = 1.0, set once
            nc.gpsimd.memset(bt0[:, 56 * NW:(TROWS - 3) * NW], 1.0)
            vp3 = bufs["vp"][0][:].rearrange("p (h w) -> p h w", w=VPW)
            nc.gpsimd.memset(vp3[:, :, 0:BLK - 1], 1.0)
            nc.gpsimd.memset(vp3[:, :, W:VPW], 1.0)
            # horizontal-chain columns that only ever see vp pads are
            # constant 1.0: a2 cols 0..2 and 56..57, b2 col 0.  (Phase 2
            # reuses a2/b2 as mask scratch and overwrites these; that only
            # affects data values of the timing-only repeat>1 builds.)
            a23i = bufs["a2"][0][:].rearrange("p (h w) -> p h w", w=VPW)
            nc.gpsimd.memset(a23i[:, :, 0:3], 1.0)
            nc.gpsimd.memset(a23i[:, :, 56:58], 1.0)
            b23i = bufs["b2"][0][:].rearrange("p (h w) -> p h w", w=VPW)
            nc.gpsimd.memset(b23i[:, :, 0:1], 1.0)

            for rep in range(repeat):
                _emit_once(nc, tc, noise_ap, x_ap, out_ap, bufs, dmstore,
                           xbstore, ones_sb, partials, gbias, stats_pool,
                           psum_pool, dram_pool, single_core or no_cc, rep)

    nc.compile()
    return nc


def _emit_once(nc, tc, noise_ap, x_ap, out_ap, bufs, dmstore, xbstore,
               ones_sb, partials, gbias, stats_pool, psum_pool, dram_pool,
               single_core, rep):
    nraws, tbs, xts = bufs["nraw"], bufs["tb"], bufs["x"]
    a, bt, vp, a2, b2 = (bufs["a"][0], bufs["bt"][0], bufs["vp"][0],
                         bufs["a2"][0], bufs["b2"][0])
    junk = bufs["junk"][0]
    vp3 = vp[:].rearrange("p (h w) -> p h w", w=VPW)

    def load_noise(t):
        nraw = nraws[t % 2]
        if t == 0:
            # halve the cold-start DMA latency: the first q half can begin
            # as soon as rows 0..25 have landed
            nc.sync.dma_start(nraw[:, 0:NPIX // 2],
                              noise_ap[bass.ts(t, P), 0:NPIX // 2])
            nc.sync.dma_start(nraw[:, NPIX // 2:NPIX],
                              noise_ap[bass.ts(t, P), NPIX // 2:NPIX])
        else:
            nc.sync.dma_start(nraw[:], noise_ap[bass.ts(t, P), :])

    def load_x(t):
        # same (sync) queue as the noise loads, issued after the noise, so
        # the compute-critical noise tiles are never stuck behind an x
        # transfer; the ring's WAR hazard throttles against the bf16 cast.
        nc.sync.dma_start(xts[t % X_BUFS][:], x_ap[bass.ts(t, P), :])

    load_noise(0)
    for t in range(NTILES):
        nraw = nraws[t % 2]
        tb = tbs[t % 2]
        dm = dmstore[:, t * OPIX:(t + 1) * OPIX]
        # next tile's noise is issued here, before anything that reads it;
        # this tile's x load is issued later in the body so the noise-side
        # queue-count waits never cover an x transfer.
        if t + 1 < NTILES:
            load_noise(t + 1)
            # next tile's q' = Sign(noise - gamma_lo) in {-1,0,+1} on ACT
            # (single op; the subtract rides the bias, f32-exact inside
            # ACT).  Min-pooling +-1 values gives -1 iff any corner drops.
            tn = tbs[(t + 1) % 2]
            trows = tn[:, (BLK - 1) * NW:(NH + BLK - 1) * NW]
            nc.scalar.activation(trows, nraws[(t + 1) % 2][:], SIGN,
                                 bias=gbias[:, 0:1])

        if t == 0:
            # tile 0's d = noise - gamma_lo runs on DVE itself (TS 2x mode,
            # f32->bf16; sign-exact), in two halves chasing the two DMA
            # halves.  Min-pooled reals and +-1s behave identically under
            # the (dm > 0) mask and Sign count.
            nc.vector.tensor_scalar(
                tb[:, (BLK - 1) * NW:30 * NW], nraw[:, 0:NPIX // 2],
                -GAMMA_LO, None, ADD)
            nc.vector.tensor_scalar(
                tb[:, 30 * NW:(NH + BLK - 1) * NW], nraw[:, NPIX // 2:NPIX],
                -GAMMA_LO, None, ADD)

        # vertical window product, log-step: windows of 2, 4, then 5.
        # A rows 0..2 and 56..58 are products of pad rows only (= 1.0,
        # pre-set); only the data-dependent rows 3..55 are computed.
        if t == 0:
            # A rows 3..28 need only T rows 3..29 (first q half)
            nc.vector.tensor_tensor(
                a[:, 3 * NW:29 * NW], tb[:, 3 * NW:29 * NW],
                tb[:, 4 * NW:30 * NW], MIN)
            nc.vector.tensor_tensor(
                a[:, 29 * NW:56 * NW], tb[:, 29 * NW:56 * NW],
                tb[:, 30 * NW:57 * NW], MIN)
        else:
            nc.vector.tensor_tensor(
                a[:, 3 * NW:56 * NW], tb[:, 3 * NW:56 * NW],
                tb[:, 4 * NW:57 * NW], MIN)
        nc.vector.tensor_tensor(
            bt[:, NW:56 * NW], a[:, NW:56 * NW],
            a[:, 3 * NW:58 * NW], MIN)
        # V[r] = B[r] * T[r+4], r in 0..55 -> into padded Vp cols 4..55
        bt3 = bt[:].rearrange("p (h w) -> p h w", w=NW)
        tb3 = tb[:].rearrange("p (h w) -> p h w", w=NW)
        nc.vector.tensor_tensor(
            vp3[:, :, BLK - 1:BLK - 1 + NW], bt3[:, 0:H, :],
            tb3[:, BLK - 1:TROWS, :], MIN)

        # horizontal window product, log-step on strided row APs so only
        # the needed columns are processed (a2 cols 0..57, b2 cols 0..55)
        a23 = a2[:].rearrange("p (h w) -> p h w", w=VPW)
        b23 = b2[:].rearrange("p (h w) -> p h w", w=VPW)
        dm3 = dm.rearrange("p (h w) -> p h w", w=W)
        nc.vector.tensor_tensor(
            a23[:, :, 3:56], vp3[:, :, 3:56], vp3[:, :, 4:57], MIN)
        nc.vector.tensor_tensor(
            b23[:, :, 1:W], a23[:, :, 1:W], a23[:, :, 3:58], MIN)
        nc.vector.tensor_tensor(
            dm3[:, :, :], b23[:, :, 0:W], vp3[:, :, BLK - 1:VPW], MIN)

        load_x(t)
        # x cast to bf16 on ACT, freeing the f32 staging slot for t+2
        nc.scalar.activation(xbstore[:, t * OPIX:(t + 1) * OPIX],
                             xts[t % X_BUFS][:], IDENT)
        # dm in {-1,+1} (tile 0: reals); keep <=> dm > 0.  ACT accumulates
        # S = sum(Sign(dm)) = #pos - #neg per partition; the count is
        # (OPIX + S)/2 (exact-zero dm are measure-zero, error << 2e-2).
        # Tile 7's count is the only one on the critical path: ACT signs
        # half of it while DVE add-reduces the other (+-1) half.
        if t < NTILES - 1:
            nc.scalar.activation(junk[:], dm, SIGN,
                                 accum_out=partials[:, t:t + 1])
        else:
            nc.scalar.activation(junk[:, 0:OPIX // 2], dm[:, 0:OPIX // 2],
                                 SIGN, accum_out=partials[:, t:t + 1])

    # ------------- global count -> scale = M / count_ones -------------
    # partials[0:7] are reduced while tile 7 is still computing; only the
    # tiny add of tile 7's count sits on the critical chain after its
    # ACT accumulation.
    p7b = stats_pool.tile([P, 1], F32, name=f"p7b{rep}", tag="p7b")
    nc.vector.tensor_reduce(
        p7b[:], dmstore[:, (NTILES - 1) * OPIX + OPIX // 2:NTILES * OPIX],
        mybir.AxisListType.X, ADD)
    phead = stats_pool.tile([P, 1], F32, name=f"phead{rep}", tag="phead")
    nc.vector.tensor_reduce(phead[:], partials[:, 0:NTILES - 1],
                            mybir.AxisListType.X, ADD)
    ptot = stats_pool.tile([P, 1], F32, name=f"ptot{rep}", tag="ptot")
    # ptot = S_partition + NTILES*OPIX: summed over partitions and cores
    # this yields S_global + M, so scale = M/count = 2M/(M+S_global).
    nc.vector.scalar_tensor_tensor(
        ptot[:], partials[:, NTILES - 1:NTILES], float(NTILES * OPIX),
        phead[:], ADD, ADD)
    ptot2 = stats_pool.tile([P, 1], F32, name=f"ptot2{rep}", tag="ptot2")
    nc.vector.tensor_tensor(ptot2[:], ptot[:], p7b[:], ADD)
    # cross-partition reduce + broadcast in one idle-PE matmul:
    # psum[m, 0] = sum_p ones[p, m] * ptot[p, 0] = total, for every m.
    ptot_ps = psum_pool.tile([P, 1], F32, name=f"ptot_ps{rep}", tag="pps")
    nc.tensor.matmul(ptot_ps[:], ones_sb[:], ptot2[:], start=True, stop=True)
    pall = stats_pool.tile([P, 1], F32, name=f"pall{rep}", tag="pall")
    nc.vector.tensor_copy(pall[:], ptot_ps[:])
    if single_core:
        tot_sb = pall
    else:
        cc_in = dram_pool.tile([P, 1], F32, name=f"cc_in{rep}", tag="cc_in")
        cc_out = dram_pool.tile([P, 1], F32, name=f"cc_out{rep}",
                                tag="cc_out")
        nc.sync.dma_start(cc_in[:], pall[:])
        nc.gpsimd.collective_compute(
            "AllReduce", ADD,
            replica_groups=[list(range(N_CORES))],
            ins=[cc_in.opt()], outs=[cc_out.opt()])
        tot_sb = stats_pool.tile([P, 1], F32, name=f"tot{rep}", tag="tot")
        nc.sync.dma_start(tot_sb[:], cc_out[:])
    recip = stats_pool.tile([P, 1], F32, name=f"recip{rep}", tag="recip")
    nc.vector.reciprocal(recip[:], tot_sb[:])
    scale_sb = stats_pool.tile([P, 1], F32, name=f"scale{rep}", tag="scale")
    nc.vector.tensor_scalar_mul(scale_sb[:], recip[:], 2.0 * COUNT_M)

    # ------- phase 2: out = xb * (mask * scale), store bf16 -------
    for t in range(NTILES):
        dm = dmstore[:, t * OPIX:(t + 1) * OPIX]
        xb = xbstore[:, t * OPIX:(t + 1) * OPIX]
        # scaled mask into a free work buffer (a2/b2 alternate), 4x mode
        msc = (a2 if t % 2 == 0 else b2)[:, 0:OPIX]
        nc.vector.tensor_scalar(msc, dm, 0.0, scale_sb[:, 0:1],
                                mybir.AluOpType.is_gt, MULT)
        nc.vector.tensor_tensor(xb, xb, msc, MULT)
        # alternate stores across both HWDGE queues so the final drain is
        # paced by aggregate DMA BW, not one queue's serialization
        eng = nc.sync if t % 2 == 0 else nc.scalar
        eng.dma_start(out_ap[bass.ts(t, P), :], xb)


def _get_nc():
    if "nc" not in _CACHE:
        _CACHE["nc"] = _build()
    return _CACHE["nc"]


def kernel(x: np.ndarray, noise: np.ndarray) -> np.ndarray:
    x = np.asarray(x, dtype=np.float32)
    noise = np.asarray(noise, dtype=np.float32)
    assert x.shape == (B, C, H, W) and noise.shape == (B, C, NH, NW)
    nc = _get_nc()
    in_maps = []
    for i in range(N_CORES):
        xs = np.ascontiguousarray(x[i * B_SH:(i + 1) * B_SH]).reshape(
            IMGS, OPIX)
        ns = np.ascontiguousarray(noise[i * B_SH:(i + 1) * B_SH]).reshape(
            IMGS, NPIX)
        in_maps.append({"x": xs, "noise": ns})
    res = run_bass_kernel_spmd(nc, in_maps, list(range(N_CORES)))
    out = np.empty((B, C, H, W), dtype=np.float32)
    for i in range(N_CORES):
        out[i * B_SH:(i + 1) * B_SH] = np.asarray(
            res.results[i]["out"]).astype(np.float32).reshape(
                B_SH, C, H, W)
    return out



# revision 49
# speedup vs baseline: 1.0094x; 1.0094x over previous
"""DropBlock kernel for Trainium2, 8 NeuronCores, batch-sharded data parallel.

Reference computation (B,C,H,W = 128,64,56,56, block=5, gamma=0.02):
    mask    = (noise < gamma)                       # (B,C,52,52) corner drops
    dilated = maxpool5x5_full_pad(mask)             # (B,C,56,56)
    block_mask = 1 - dilated
    out = block_mask * x * (numel / sum(block_mask))

Schedule:
  - The eight noise tiles stream first on the SP HWDGE queue (f32 — the
    gamma compare must be f32-exact).  ACT runs only Sign ops during this
    window (the mask-relus are pinned behind all Signs via an inert bias
    read of a marker written from Sign_7's output), so the 3-slot noise
    ring never throttles the stream.
  - x is loaded by TWO gpsimd (SWDGE) cast-DMAs, f32 DRAM -> bf16 SBUF,
    4 tiles each (DRAM-side AP exposes the tile dim so row t*P+p lands
    on partition p): the DMA bus cost is charged at the bf16 size,
    halving the x read time, and no ACT cast phase exists at all.  Each
    chunk is release-gated by a 1-element DVE tensor_copy reading a
    mid-pipeline chain scratch, so the x transfers queue up behind the
    noise stream instead of racing it for DMA bandwidth.
  - The 5x5 window-min chain (log-step, 6 tensor_tensor ops/tile on +-1
    signs) runs entirely on DVE (2x bf16): neuronxcc rejects
    TensorTensor/ScalarTensorTensor on GpSimd, and ACT/PE have no
    elementwise two-tensor path, so DVE throughput is the kernel's
    floor.  Tile 0's vertical chain is split in halves chasing the two
    Sign_0 halves; tile 7's horizontal tail and mask-relu are split in
    row-halves so the collective-critical count lands early.
  - Masks+counts: ACT Relu in place doubles as the +-1 -> {0,1}
    conversion and the exact per-tile count (accum_out).  (The DVE
    tensor_scalar accumulator writes zeros through neuronxcc/birsim —
    counts must stay on ACT.)
  - Tail per tile (scale-gated, so the static scheduler cannot hoist it
    into the chain window): out = mask*scale (DVE 4x tensor_scalar, or
    Pool tensor_scalar / ACT per-partition-scale activation for three
    tiles each, keeping the multiplies off saturated DVE) then
    out *= xb (DVE 2x).  Tile 0 is processed in quarter/half pieces so
    the DMA-bound store stream starts early; stores alternate across
    the SP and ACT HWDGE queues.
  - scale = COUNT_M / count_global; cross-partition reduce+broadcast is
    one idle-PE matmul against ones; cross-core is a [P,1] AllReduce on
    the ACT HWDGE queue (the SP queue still has x/store traffic).

Exactness: q = Sign(noise - gamma_lo) with the subtract riding the ACT
bias (f32 inside the datapath); q=0 at noise==gamma_lo propagates through
min into mask=0, matching the reference drop.  Counts are integer-exact
in f32; the only roundings are the bf16 cast of x and the bf16 store.
"""

import sys

sys.path.insert(0, "/opt/trn_rl_repo")

import numpy as np

import concourse.bacc as bacc
import concourse.bass as bass
import concourse.tile as tile
import concourse.mybir as mybir
from concourse.bass_utils import run_bass_kernel_spmd

N_CORES = 8
B, C, H, W = 128, 64, 56, 56
BLK = 5
GAMMA = 0.02
NH, NW = H - (BLK - 1), W - (BLK - 1)  # 52, 52 noise dims
B_SH = B // N_CORES  # 16 batches per core
IMGS = B_SH * C  # 1024 images per core
P = 128  # partitions
NTILES = IMGS // P  # 8 tiles per core
NPIX = NH * NW  # 2704 noise pixels/image
OPIX = H * W  # 3136 out pixels/image
TROWS = NH + 2 * (BLK - 1)  # 60 rows in padded vertical buffer
TFLAT = TROWS * NW  # 3120
VPW = NW + 2 * (BLK - 1)  # 60 cols in padded horizontal buffer (4+52+4)
COUNT_M = float(B * C * H * W)  # 25690112.0

# Largest f32 strictly below 0.02f: noise > gamma_lo  <=>  noise >= 0.02f.
GAMMA_LO = float(np.nextafter(np.float32(GAMMA), np.float32(0)))

F32 = mybir.dt.float32
BF16 = mybir.dt.bfloat16
MIN = mybir.AluOpType.min
MULT = mybir.AluOpType.mult
ADD = mybir.AluOpType.add
IS_GT = mybir.AluOpType.is_gt
IDENT = mybir.ActivationFunctionType.Identity
SIGN = mybir.ActivationFunctionType.Sign
RELU = mybir.ActivationFunctionType.Relu

N_BUFS = 3  # noise f32 staging ring
# ALL masks+counts ride ACT Relu with accum_out: the DVE tensor_scalar
# accumulator silently writes zeros through neuronxcc/birsim, so no
# counts may live on DVE
DVE_CNT = ()
# tail engine split: Pool tiles get msc = mask*scale via tensor_scalar
# (the only elementwise op neuronxcc allows on GpSimd), ACT tiles get
# xs = xb*scale via activation with a per-partition scale — both keep
# the scale-multiplies off DVE, which the chain already saturates
POOL_TAIL = (1, 3, 5)
ACT_TAIL = (4, 6, 7)
# x cast-load chunks: (first_tile, n_tiles, gate_tile) — the transfer is
# released by a tiny DVE copy that reads gate_tile's a2 chain scratch
# (DVE-local, so the gate never makes DVE wait on another engine)
X_CHUNKS = ((0, 4, 2), (4, 4, 4))

_CACHE = {}


def _build(single_core=False, repeat=1, no_cc=False):
    """Build + compile the SPMD bass module once.

    single_core=True builds a collective-free variant (the per-core count is
    used directly as the global count) for cost-model simulation only.
    repeat>1 unrolls the whole pipeline k times (benchmarking only).
    no_cc=True skips the AllReduce on the 8-core build (timing probe only —
    results are wrong by the per-core/global count ratio).
    """
    nc = bacc.Bacc("TRN2", target_bir_lowering=False, debug=False,
                   num_devices=1 if single_core else N_CORES)
    noise_ap = nc.dram_tensor("noise", [IMGS, NPIX], F32,
                              kind="ExternalInput").ap()
    x_ap = nc.dram_tensor("x", [IMGS, OPIX], F32, kind="ExternalInput").ap()
    out_ap = nc.dram_tensor("out", [IMGS, OPIX], BF16,
                            kind="ExternalOutput").ap()

    with tile.TileContext(nc) as tc:
        with (
            tc.tile_pool(name="buf", bufs=1) as bp,
            tc.tile_pool(name="stats", bufs=1) as stats_pool,
            tc.tile_pool(name="ps", bufs=1, space="PSUM") as psum_pool,
            tc.tile_pool(name="dram", bufs=1, space="DRAM") as dram_pool,
        ):
            bufs = {
                "nraw": [bp.tile([P, NPIX], F32, name=f"nraw{s}")
                         for s in range(N_BUFS)],
                "tb": [bp.tile([P, TFLAT], BF16, name=f"tb{s}")
                       for s in range(2)],
                "a": [bp.tile([P, (TROWS - 1) * NW], BF16, name="a0")],
                "bt": [bp.tile([P, (TROWS - 3) * NW], BF16, name="bt0")],
                "vp": [bp.tile([P, H * VPW], BF16, name=f"vp{s}")
                       for s in range(2)],
                "a2": [bp.tile([P, H * VPW], BF16, name=f"a2_{s}")
                       for s in range(2)],
                "b2": [bp.tile([P, H * VPW], BF16, name=f"b2_{s}")
                       for s in range(3)],
            }
            dmstore = bp.tile([P, NTILES * OPIX], BF16, name="dmstore")
            xbstore = bp.tile([P, NTILES * OPIX], BF16, name="xbstore")
            ones_sb = bp.tile([P, P], F32, name="ones")
            nc.gpsimd.memset(ones_sb[:], 1.0)
            # one extra slot: tile 7's count arrives as two half-relus
            partials = stats_pool.tile([P, NTILES + 1], F32)
            gbias = stats_pool.tile([P, 1], F32)
            nc.vector.memset(gbias[:], -GAMMA_LO)
            # warm the ACT function tables on a 1-element tile during the
            # DMA lead-in; otherwise LoadActFuncSet (~1.3us) lands in front
            # of the first real op and stalls the pipeline.
            warm = stats_pool.tile([P, 1], F32)
            nc.scalar.activation(warm[:], gbias[:], IDENT, bias=gbias[:, 0:1])
            nc.scalar.activation(warm[:], warm[:], SIGN)
            nc.scalar.activation(warm[:], warm[:], RELU)

            # 1.0-pads written once; every later op touches only data rows.
            for tb in bufs["tb"]:
                nc.gpsimd.memset(tb[:, 0:(BLK - 1) * NW], 1.0)
                nc.gpsimd.memset(tb[:, (NH + BLK - 1) * NW:TFLAT], 1.0)
            a0 = bufs["a"][0]
            nc.gpsimd.memset(a0[:, 0:3 * NW], 1.0)
            nc.gpsimd.memset(a0[:, 56 * NW:(TROWS - 1) * NW], 1.0)
            bt0 = bufs["bt"][0]
            nc.gpsimd.memset(bt0[:, 0:NW], 1.0)
            nc.gpsimd.memset(bt0[:, 56 * NW:(TROWS - 3) * NW], 1.0)
            for vp in bufs["vp"]:
                vp3 = vp[:].rearrange("p (h w) -> p h w", w=VPW)
                nc.gpsimd.memset(vp3[:, :, 0:BLK - 1], 1.0)
                nc.gpsimd.memset(vp3[:, :, W:VPW], 1.0)
            # horizontal-chain columns that only ever see vp pads are
            # constant 1.0: a2 cols 0..2 and 56..57, b2 col 0.
            for a2 in bufs["a2"]:
                a23i = a2[:].rearrange("p (h w) -> p h w", w=VPW)
                nc.gpsimd.memset(a23i[:, :, 0:3], 1.0)
                nc.gpsimd.memset(a23i[:, :, 56:58], 1.0)
            for b2 in bufs["b2"]:
                b23i = b2[:].rearrange("p (h w) -> p h w", w=VPW)
                nc.gpsimd.memset(b23i[:, :, 0:1], 1.0)

            for rep in range(repeat):
                _emit_once(nc, tc, noise_ap, x_ap, out_ap, bufs, dmstore,
                           xbstore, ones_sb, partials, gbias, stats_pool,
                           psum_pool, dram_pool, single_core or no_cc, rep)

    nc.compile()
    return nc


def _emit_once(nc, tc, noise_ap, x_ap, out_ap, bufs, dmstore, xbstore,
               ones_sb, partials, gbias, stats_pool, psum_pool, dram_pool,
               single_core, rep):
    nraws, tbs = bufs["nraw"], bufs["tb"]
    a, bt = bufs["a"][0], bufs["bt"][0]

    def dms(t):
        return dmstore[:, t * OPIX:(t + 1) * OPIX]

    def xbs(t):
        return xbstore[:, t * OPIX:(t + 1) * OPIX]

    def load_noise(t):
        nraw = nraws[t % N_BUFS]
        if t == 0:
            # halve the cold-start DMA latency: Sign_0 on the first half
            # can begin as soon as rows 0..25 have landed
            nc.sync.dma_start(nraw[:, 0:NPIX // 2],
                              noise_ap[bass.ts(t, P), 0:NPIX // 2])
            nc.sync.dma_start(nraw[:, NPIX // 2:NPIX],
                              noise_ap[bass.ts(t, P), NPIX // 2:NPIX])
        else:
            nc.sync.dma_start(nraw[:], noise_ap[bass.ts(t, P), :])

    def load_x_chunk(t0, n, gate_t):
        # release gate: a 1-element DVE copy that reads tile gate_t's a2
        # scratch — the cast-DMA (WAW on xbstore) cannot start, and
        # therefore cannot steal DMA bandwidth from the noise stream,
        # before the chain pipeline has reached gate_t.  a2 is DVE-local,
        # so the gate costs DVE nothing.
        # read a DATA column of a2 (col 3) — cols 0..2 are pad constants
        # written once at setup, which would release the gate immediately
        nc.vector.tensor_copy(xbstore[:, t0 * OPIX:t0 * OPIX + 1],
                              bufs["a2"][gate_t % 2][:, 3:4])
        # DRAM row t*P+p must land on partition p, tile-block t: expose
        # the tile dim as a middle AP dim on both sides
        src = x_ap[t0 * P:(t0 + n) * P, :].rearrange(
            "(t p) c -> p t c", p=P)
        dst = xbstore[:, t0 * OPIX:(t0 + n) * OPIX].rearrange(
            "p (t c) -> p t c", c=OPIX)
        nc.gpsimd.dma_start(dst, src)

    def sign(t):
        tn = tbs[t % 2]
        trows = tn[:, (BLK - 1) * NW:(NH + BLK - 1) * NW]
        nraw = nraws[t % N_BUFS]
        if t == 0:
            half = (BLK - 1) * NW + (NPIX // 2)
            nc.scalar.activation(tn[:, (BLK - 1) * NW:half],
                                 nraw[:, 0:NPIX // 2], SIGN,
                                 bias=gbias[:, 0:1])
            nc.scalar.activation(tn[:, half:(NH + BLK - 1) * NW],
                                 nraw[:, NPIX // 2:NPIX], SIGN,
                                 bias=gbias[:, 0:1])
        else:
            nc.scalar.activation(trows, nraw[:], SIGN, bias=gbias[:, 0:1])

    # --------------- window-min chain, DVE/Pool split ---------------
    # vertical log-step on tb (pads preset): a = win2, bt = win4,
    # vp = win5 (into padded cols 4..55); horizontal mirror on vp.
    def chain(t):
        tb = tbs[t % 2]
        vp, a2, b2 = (bufs["vp"][t % 2], bufs["a2"][t % 2],
                      bufs["b2"][t % 3])
        vp3 = vp[:].rearrange("p (h w) -> p h w", w=VPW)
        a23 = a2[:].rearrange("p (h w) -> p h w", w=VPW)
        b23 = b2[:].rearrange("p (h w) -> p h w", w=VPW)
        dm3 = dms(t).rearrange("p (h w) -> p h w", w=W)

        bt3 = bt[:].rearrange("p (h w) -> p h w", w=NW)
        tb3 = tb[:].rearrange("p (h w) -> p h w", w=NW)
        if t == 0:
            # tile 0's vertical chain in lo/hi halves chasing the two
            # halves of Sign_0 (the second noise-DMA half lands ~2us
            # after the first): lo needs only tb rows <= 29
            nc.vector.tensor_tensor(
                a[:, 3 * NW:29 * NW], tb[:, 3 * NW:29 * NW],
                tb[:, 4 * NW:30 * NW], MIN)
            nc.vector.tensor_tensor(
                bt[:, NW:27 * NW], a[:, NW:27 * NW],
                a[:, 3 * NW:29 * NW], MIN)
            nc.vector.tensor_tensor(
                vp3[:, 0:23, BLK - 1:BLK - 1 + NW], bt3[:, 0:23, :],
                tb3[:, BLK - 1:23 + BLK - 1, :], MIN)
            nc.vector.tensor_tensor(
                a[:, 29 * NW:56 * NW], tb[:, 29 * NW:56 * NW],
                tb[:, 30 * NW:57 * NW], MIN)
            nc.vector.tensor_tensor(
                bt[:, 27 * NW:56 * NW], a[:, 27 * NW:56 * NW],
                a[:, 29 * NW:58 * NW], MIN)
            nc.vector.tensor_tensor(
                vp3[:, 23:H, BLK - 1:BLK - 1 + NW], bt3[:, 23:H, :],
                tb3[:, 23 + BLK - 1:TROWS, :], MIN)
        else:
            nc.vector.tensor_tensor(
                a[:, 3 * NW:56 * NW], tb[:, 3 * NW:56 * NW],
                tb[:, 4 * NW:57 * NW], MIN)
            nc.vector.tensor_tensor(
                bt[:, NW:56 * NW], a[:, NW:56 * NW],
                a[:, 3 * NW:58 * NW], MIN)
            nc.vector.tensor_tensor(
                vp3[:, :, BLK - 1:BLK - 1 + NW], bt3[:, 0:H, :],
                tb3[:, BLK - 1:TROWS, :], MIN)
        nc.vector.tensor_tensor(
            a23[:, :, 3:56], vp3[:, :, 3:56], vp3[:, :, 4:57], MIN)
        # NOTE: neuronxcc only accepts TensorTensor on DVE — the GpSimd
        # (Pool) engine is limited to tensor_scalar/memset/DMA on the
        # real ISA, so the whole min chain lives on DVE
        if t == NTILES - 1:
            # last tile's horizontal tail in row-halves: each half's
            # mask-relu (the count the collective waits on) can start
            # ~1.7us earlier than a whole-tile op would allow
            hh = H // 2
            for r0, r1 in ((0, hh), (hh, H)):
                nc.vector.tensor_tensor(
                    b23[:, r0:r1, 1:W], a23[:, r0:r1, 1:W],
                    a23[:, r0:r1, 3:58], MIN)
                nc.vector.tensor_tensor(
                    dm3[:, r0:r1, :], b23[:, r0:r1, 0:W],
                    vp3[:, r0:r1, BLK - 1:VPW], MIN)
        else:
            nc.vector.tensor_tensor(
                b23[:, :, 1:W], a23[:, :, 1:W], a23[:, :, 3:58], MIN)
            nc.vector.tensor_tensor(
                dm3[:, :, :], b23[:, :, 0:W], vp3[:, :, BLK - 1:VPW], MIN)

    def mask_dve(t):
        # explicit 2nd op: the ISA's TensorScalarPtrReduce (accum_out)
        # requires both ALU stages
        nc.vector.tensor_scalar(dms(t), dms(t), 0.0, 1.0, IS_GT, MULT,
                                accum_out=partials[:, t:t + 1])

    # smark: a [P,1] zero written via a read of Sign_7's output — every
    # ACT mask-relu takes it as (numerically inert) bias, which pins the
    # relus BEHIND all eight Signs in the compiled ACT order.  Without
    # this the list scheduler interleaves relus between the Signs, and
    # each inversion stalls the noise ring by ~3us.
    smark = stats_pool.tile([P, 1], F32, name=f"smark{rep}", tag="smark")

    def mask_relu(t):
        nc.scalar.activation(dms(t), dms(t), RELU, bias=smark[:, 0:1],
                             accum_out=partials[:, t:t + 1])

    # noise loads + signs just-in-time; chain(t) before sign(t+2) so the
    # tb ring WAR resolves correctly.  The ACT mask-relus are emitted only
    # AFTER the last sign so the compiled ACT order keeps every Sign (which
    # gates the noise ring and the chains) ahead of them.
    load_noise(0)
    sign(0)
    load_noise(1)
    sign(1)
    for t in range(NTILES - 2):
        chain(t)
        if t in DVE_CNT:
            mask_dve(t)
        if t == X_CHUNKS[0][2]:
            load_x_chunk(*X_CHUNKS[0])
        elif t == X_CHUNKS[1][2]:
            load_x_chunk(*X_CHUNKS[1])
        load_noise(t + 2)
        sign(t + 2)
    # smark = (a data element of Sign_7's tb slot) * 0.0
    nc.vector.tensor_scalar(
        smark[:], tbs[(NTILES - 1) % 2][:, (BLK - 1) * NW:(BLK - 1) * NW + 1],
        0.0, None, MULT)
    for t in range(NTILES - 2):
        mask_relu(t)
    chain(NTILES - 2)
    mask_relu(NTILES - 2)
    chain(NTILES - 1)
    # tile 7's mask-relu in halves (matching the split chain tail) so
    # the collective-critical count completes as early as possible
    t7 = (NTILES - 1) * OPIX
    hp = (H // 2) * W
    nc.scalar.activation(dmstore[:, t7:t7 + hp],
                         dmstore[:, t7:t7 + hp], RELU,
                         bias=smark[:, 0:1],
                         accum_out=partials[:, NTILES - 1:NTILES])
    nc.scalar.activation(dmstore[:, t7 + hp:t7 + OPIX],
                         dmstore[:, t7 + hp:t7 + OPIX], RELU,
                         bias=smark[:, 0:1],
                         accum_out=partials[:, NTILES:NTILES + 1])

    # ------------- global count -> scale = M / count_ones -------------
    ptot = stats_pool.tile([P, 1], F32, name=f"ptot{rep}", tag="ptot")
    nc.vector.tensor_reduce(ptot[:], partials[:, 0:NTILES + 1],
                            mybir.AxisListType.X, ADD)
    # cross-partition reduce + broadcast in one idle-PE matmul:
    # psum[m, 0] = sum_p ones[p, m] * ptot[p, 0] = total, for every m.
    ptot_ps = psum_pool.tile([P, 1], F32, name=f"ptot_ps{rep}", tag="pps")
    nc.tensor.matmul(ptot_ps[:], ones_sb[:], ptot[:], start=True, stop=True)
    pall = stats_pool.tile([P, 1], F32, name=f"pall{rep}", tag="pall")
    nc.vector.tensor_copy(pall[:], ptot_ps[:])
    if single_core:
        tot_sb = pall
    else:
        # cc transfers ride the scalar HWDGE queue: the sync queue's FIFO
        # may still hold store traffic
        cc_in = dram_pool.tile([P, 1], F32, name=f"cc_in{rep}", tag="cc_in")
        cc_out = dram_pool.tile([P, 1], F32, name=f"cc_out{rep}",
                                tag="cc_out")
        nc.scalar.dma_start(cc_in[:], pall[:])
        nc.gpsimd.collective_compute(
            "AllReduce", ADD,
            replica_groups=[list(range(N_CORES))],
            ins=[cc_in.opt()], outs=[cc_out.opt()])
        tot_sb = stats_pool.tile([P, 1], F32, name=f"tot{rep}", tag="tot")
        nc.scalar.dma_start(tot_sb[:], cc_out[:])
    recip = stats_pool.tile([P, 1], F32, name=f"recip{rep}", tag="recip")
    nc.vector.reciprocal(recip[:], tot_sb[:])
    scale_sb = stats_pool.tile([P, 1], F32, name=f"scale{rep}", tag="scale")
    nc.vector.tensor_scalar_mul(scale_sb[:], recip[:], COUNT_M)

    # ------- scale-gated tail: out_t = mask_t * xb_t * scale -------
    # every tail op reads scale_sb (directly or transitively), so the
    # static scheduler cannot move any of it into the chain window
    def tail(t):
        if t in POOL_TAIL:
            # msc = mask*scale on Pool (tensor_scalar — GpSimd-legal)
            nc.gpsimd.tensor_scalar(dms(t), dms(t), scale_sb[:, 0:1],
                                    None, MULT)
            nc.vector.tensor_tensor(dms(t), dms(t), xbs(t), MULT)
        elif t in ACT_TAIL:
            # xs = xb*scale on ACT (per-partition activation scale)
            nc.scalar.activation(xbs(t), xbs(t), IDENT,
                                 scale=scale_sb[:, 0:1])
            nc.vector.tensor_tensor(dms(t), dms(t), xbs(t), MULT)
        else:
            nc.vector.tensor_scalar(dms(t), dms(t), scale_sb[:, 0:1],
                                    None, MULT)
            nc.vector.tensor_tensor(dms(t), dms(t), xbs(t), MULT)
        # alternate stores across both HWDGE queues so the final drain is
        # paced by aggregate DMA BW, not one queue's serialization
        eng = nc.sync if t % 2 == 0 else nc.scalar
        eng.dma_start(out_ap[bass.ts(t, P), :], dms(t))

    def tail0_pieces():
        # tile 0's tail in quarter/quarter/half pieces so the store
        # stream (the final DMA-bound 17.8us) starts ~4us earlier
        q = OPIX // 4
        for lo, hi in ((0, q), (q, 2 * q), (2 * q, OPIX)):
            d = dmstore[:, lo:hi]
            nc.vector.tensor_scalar(d, d, scale_sb[:, 0:1], None, MULT)
            nc.vector.tensor_tensor(d, d, xbstore[:, lo:hi], MULT)
            nc.sync.dma_start(out_ap[bass.ts(0, P), lo:hi], d)

    tail0_pieces()
    for t in range(1, NTILES):
        tail(t)


def _get_nc():
    if "nc" not in _CACHE:
        _CACHE["nc"] = _build()
    return _CACHE["nc"]


def kernel(x: np.ndarray, noise: np.ndarray) -> np.ndarray:
    x = np.asarray(x, dtype=np.float32)
    noise = np.asarray(noise, dtype=np.float32)
    assert x.shape == (B, C, H, W) and noise.shape == (B, C, NH, NW)
    nc = _get_nc()
    in_maps = []
    for i in range(N_CORES):
        xs = np.ascontiguousarray(x[i * B_SH:(i + 1) * B_SH]).reshape(
            IMGS, OPIX)
        ns = np.ascontiguousarray(noise[i * B_SH:(i + 1) * B_SH]).reshape(
            IMGS, NPIX)
        in_maps.append({"x": xs, "noise": ns})
    res = run_bass_kernel_spmd(nc, in_maps, list(range(N_CORES)))
    out = np.empty((B, C, H, W), dtype=np.float32)
    for i in range(N_CORES):
        out[i * B_SH:(i + 1) * B_SH] = np.asarray(
            res.results[i]["out"]).astype(np.float32).reshape(
                B_SH, C, H, W)
    return out


# revision 51
# speedup vs baseline: 1.0143x; 1.0048x over previous
"""DropBlock kernel for Trainium2, 8 NeuronCores, batch-sharded data parallel.

Reference computation (B,C,H,W = 128,64,56,56, block=5, gamma=0.02):
    mask    = (noise < gamma)                       # (B,C,52,52) corner drops
    dilated = maxpool5x5_full_pad(mask)             # (B,C,56,56)
    block_mask = 1 - dilated
    out = block_mask * x * (numel / sum(block_mask))

Schedule:
  - The eight noise tiles stream first on the SP HWDGE queue (f32 — the
    gamma compare must be f32-exact).  ACT runs only Sign ops during this
    window (the mask-relus are pinned behind all Signs via an inert bias
    read of a marker written from Sign_7's output), so the 3-slot noise
    ring never throttles the stream.
  - x is loaded by TWO gpsimd (SWDGE) cast-DMAs, f32 DRAM -> bf16 SBUF,
    4 tiles each (DRAM-side AP exposes the tile dim so row t*P+p lands
    on partition p): the DMA bus cost is charged at the bf16 size,
    halving the x read time, and no ACT cast phase exists at all.  Each
    chunk is release-gated by a 1-element DVE tensor_copy reading a
    mid-pipeline chain scratch, so the x transfers queue up behind the
    noise stream instead of racing it for DMA bandwidth.
  - The 5x5 window-min chain (log-step, 6 tensor_tensor ops/tile on +-1
    signs) runs entirely on DVE (2x bf16): neuronxcc rejects
    TensorTensor/ScalarTensorTensor on GpSimd, and ACT/PE have no
    elementwise two-tensor path, so DVE throughput is the kernel's
    floor.  Tile 0's vertical chain is split in halves chasing the two
    Sign_0 halves; tile 7's horizontal tail and mask-relu are split in
    row-halves so the collective-critical count lands early.
  - Masks+counts: ACT Relu in place doubles as the +-1 -> {0,1}
    conversion and the exact per-tile count (accum_out).  (The DVE
    tensor_scalar accumulator writes zeros through neuronxcc/birsim —
    counts must stay on ACT.)
  - Tail per tile (scale-gated, so the static scheduler cannot hoist it
    into the chain window): out = mask*scale (DVE 4x tensor_scalar, or
    Pool tensor_scalar / ACT per-partition-scale activation for three
    tiles each, keeping the multiplies off saturated DVE) then
    out *= xb (DVE 2x).  Tile 0 is processed in quarter/half pieces so
    the DMA-bound store stream starts early; stores alternate across
    the SP and ACT HWDGE queues.
  - scale = COUNT_M / count_global; cross-partition reduce+broadcast is
    one idle-PE matmul against ones; cross-core is a [P,1] AllReduce on
    the ACT HWDGE queue (the SP queue still has x/store traffic).

Exactness: q = Sign(noise - gamma_lo) with the subtract riding the ACT
bias (f32 inside the datapath); q=0 at noise==gamma_lo propagates through
min into mask=0, matching the reference drop.  Counts are integer-exact
in f32; the only roundings are the bf16 cast of x and the bf16 store.
"""

import sys

sys.path.insert(0, "/opt/trn_rl_repo")

import numpy as np

import concourse.bacc as bacc
import concourse.bass as bass
import concourse.tile as tile
import concourse.mybir as mybir
from concourse.bass_utils import run_bass_kernel_spmd

N_CORES = 8
B, C, H, W = 128, 64, 56, 56
BLK = 5
GAMMA = 0.02
NH, NW = H - (BLK - 1), W - (BLK - 1)  # 52, 52 noise dims
B_SH = B // N_CORES  # 16 batches per core
IMGS = B_SH * C  # 1024 images per core
P = 128  # partitions
NTILES = IMGS // P  # 8 tiles per core
NPIX = NH * NW  # 2704 noise pixels/image
OPIX = H * W  # 3136 out pixels/image
TROWS = NH + 2 * (BLK - 1)  # 60 rows in padded vertical buffer
TFLAT = TROWS * NW  # 3120
VPW = NW + 2 * (BLK - 1)  # 60 cols in padded horizontal buffer (4+52+4)
COUNT_M = float(B * C * H * W)  # 25690112.0

# Largest f32 strictly below 0.02f: noise > gamma_lo  <=>  noise >= 0.02f.
GAMMA_LO = float(np.nextafter(np.float32(GAMMA), np.float32(0)))

F32 = mybir.dt.float32
BF16 = mybir.dt.bfloat16
MIN = mybir.AluOpType.min
MULT = mybir.AluOpType.mult
ADD = mybir.AluOpType.add
IS_GT = mybir.AluOpType.is_gt
IDENT = mybir.ActivationFunctionType.Identity
SIGN = mybir.ActivationFunctionType.Sign
RELU = mybir.ActivationFunctionType.Relu

N_BUFS = 3  # noise f32 staging ring
# ALL masks+counts ride ACT Relu with accum_out: the DVE tensor_scalar
# accumulator silently writes zeros through neuronxcc/birsim, so no
# counts may live on DVE
DVE_CNT = ()
# tail engine split: Pool tiles get msc = mask*scale via tensor_scalar
# (the only elementwise op neuronxcc allows on GpSimd), ACT tiles get
# xs = xb*scale via activation with a per-partition scale — both keep
# the scale-multiplies off DVE, which the chain already saturates
POOL_TAIL = (1, 3)
ACT_TAIL = (4, 6, 7)
# x cast-load chunks: (first_tile, n_tiles, gate_tile) — the transfer is
# released by a tiny DVE copy that reads gate_tile's a2 chain scratch
# (DVE-local, so the gate never makes DVE wait on another engine)
X_CHUNKS = ((0, 4, 2), (4, 4, 4))

_CACHE = {}


def _build(single_core=False, repeat=1, no_cc=False):
    """Build + compile the SPMD bass module once.

    single_core=True builds a collective-free variant (the per-core count is
    used directly as the global count) for cost-model simulation only.
    repeat>1 unrolls the whole pipeline k times (benchmarking only).
    no_cc=True skips the AllReduce on the 8-core build (timing probe only —
    results are wrong by the per-core/global count ratio).
    """
    nc = bacc.Bacc("TRN2", target_bir_lowering=False, debug=False,
                   num_devices=1 if single_core else N_CORES)
    noise_ap = nc.dram_tensor("noise", [IMGS, NPIX], F32,
                              kind="ExternalInput").ap()
    x_ap = nc.dram_tensor("x", [IMGS, OPIX], F32, kind="ExternalInput").ap()
    out_ap = nc.dram_tensor("out", [IMGS, OPIX], BF16,
                            kind="ExternalOutput").ap()

    with tile.TileContext(nc) as tc:
        with (
            tc.tile_pool(name="buf", bufs=1) as bp,
            tc.tile_pool(name="stats", bufs=1) as stats_pool,
            tc.tile_pool(name="ps", bufs=1, space="PSUM") as psum_pool,
            tc.tile_pool(name="dram", bufs=1, space="DRAM") as dram_pool,
        ):
            bufs = {
                "nraw": [bp.tile([P, NPIX], F32, name=f"nraw{s}")
                         for s in range(N_BUFS)],
                "tb": [bp.tile([P, TFLAT], BF16, name=f"tb{s}")
                       for s in range(2)],
                "a": [bp.tile([P, (TROWS - 1) * NW], BF16, name="a0")],
                "bt": [bp.tile([P, (TROWS - 3) * NW], BF16, name="bt0")],
                "vp": [bp.tile([P, H * VPW], BF16, name=f"vp{s}")
                       for s in range(2)],
                "a2": [bp.tile([P, H * VPW], BF16, name=f"a2_{s}")
                       for s in range(2)],
                "b2": [bp.tile([P, H * VPW], BF16, name=f"b2_{s}")
                       for s in range(3)],
            }
            dmstore = bp.tile([P, NTILES * OPIX], BF16, name="dmstore")
            xbstore = bp.tile([P, NTILES * OPIX], BF16, name="xbstore")
            ones_sb = bp.tile([P, P], F32, name="ones")
            nc.gpsimd.memset(ones_sb[:], 1.0)
            # one extra slot: tile 7's count arrives as two half-relus
            partials = stats_pool.tile([P, NTILES + 1], F32)
            gbias = stats_pool.tile([P, 1], F32)
            nc.vector.memset(gbias[:], -GAMMA_LO)
            # warm the ACT function tables on a 1-element tile during the
            # DMA lead-in; otherwise LoadActFuncSet (~1.3us) lands in front
            # of the first real op and stalls the pipeline.
            warm = stats_pool.tile([P, 1], F32)
            nc.scalar.activation(warm[:], gbias[:], IDENT, bias=gbias[:, 0:1])
            nc.scalar.activation(warm[:], warm[:], SIGN)
            nc.scalar.activation(warm[:], warm[:], RELU)

            # 1.0-pads written once; every later op touches only data rows.
            for tb in bufs["tb"]:
                nc.gpsimd.memset(tb[:, 0:(BLK - 1) * NW], 1.0)
                nc.gpsimd.memset(tb[:, (NH + BLK - 1) * NW:TFLAT], 1.0)
            a0 = bufs["a"][0]
            nc.gpsimd.memset(a0[:, 0:3 * NW], 1.0)
            nc.gpsimd.memset(a0[:, 56 * NW:(TROWS - 1) * NW], 1.0)
            bt0 = bufs["bt"][0]
            nc.gpsimd.memset(bt0[:, 0:NW], 1.0)
            nc.gpsimd.memset(bt0[:, 56 * NW:(TROWS - 3) * NW], 1.0)
            for vp in bufs["vp"]:
                vp3 = vp[:].rearrange("p (h w) -> p h w", w=VPW)
                nc.gpsimd.memset(vp3[:, :, 0:BLK - 1], 1.0)
                nc.gpsimd.memset(vp3[:, :, W:VPW], 1.0)
            # horizontal-chain columns that only ever see vp pads are
            # constant 1.0: a2 cols 0..2 and 56..57, b2 col 0.
            for a2 in bufs["a2"]:
                a23i = a2[:].rearrange("p (h w) -> p h w", w=VPW)
                nc.gpsimd.memset(a23i[:, :, 0:3], 1.0)
                nc.gpsimd.memset(a23i[:, :, 56:58], 1.0)
            for b2 in bufs["b2"]:
                b23i = b2[:].rearrange("p (h w) -> p h w", w=VPW)
                nc.gpsimd.memset(b23i[:, :, 0:1], 1.0)

            for rep in range(repeat):
                _emit_once(nc, tc, noise_ap, x_ap, out_ap, bufs, dmstore,
                           xbstore, ones_sb, partials, gbias, stats_pool,
                           psum_pool, dram_pool, single_core or no_cc, rep)

    nc.compile()
    return nc


def _emit_once(nc, tc, noise_ap, x_ap, out_ap, bufs, dmstore, xbstore,
               ones_sb, partials, gbias, stats_pool, psum_pool, dram_pool,
               single_core, rep):
    nraws, tbs = bufs["nraw"], bufs["tb"]
    a, bt = bufs["a"][0], bufs["bt"][0]

    def dms(t):
        return dmstore[:, t * OPIX:(t + 1) * OPIX]

    def xbs(t):
        return xbstore[:, t * OPIX:(t + 1) * OPIX]

    def load_noise(t):
        nraw = nraws[t % N_BUFS]
        if t == 0:
            # halve the cold-start DMA latency: Sign_0 on the first half
            # can begin as soon as rows 0..25 have landed
            nc.sync.dma_start(nraw[:, 0:NPIX // 2],
                              noise_ap[bass.ts(t, P), 0:NPIX // 2])
            nc.sync.dma_start(nraw[:, NPIX // 2:NPIX],
                              noise_ap[bass.ts(t, P), NPIX // 2:NPIX])
        else:
            nc.sync.dma_start(nraw[:], noise_ap[bass.ts(t, P), :])

    def load_x_chunk(t0, n, gate_t):
        # release gate: a 1-element DVE copy that reads tile gate_t's a2
        # scratch — the cast-DMA (WAW on xbstore) cannot start, and
        # therefore cannot steal DMA bandwidth from the noise stream,
        # before the chain pipeline has reached gate_t.  a2 is DVE-local,
        # so the gate costs DVE nothing.
        # read a DATA column of a2 (col 3) — cols 0..2 are pad constants
        # written once at setup, which would release the gate immediately
        nc.vector.tensor_copy(xbstore[:, t0 * OPIX:t0 * OPIX + 1],
                              bufs["a2"][gate_t % 2][:, 3:4])
        # DRAM row t*P+p must land on partition p, tile-block t: expose
        # the tile dim as a middle AP dim on both sides
        src = x_ap[t0 * P:(t0 + n) * P, :].rearrange(
            "(t p) c -> p t c", p=P)
        dst = xbstore[:, t0 * OPIX:(t0 + n) * OPIX].rearrange(
            "p (t c) -> p t c", c=OPIX)
        nc.gpsimd.dma_start(dst, src)

    def sign(t):
        tn = tbs[t % 2]
        trows = tn[:, (BLK - 1) * NW:(NH + BLK - 1) * NW]
        nraw = nraws[t % N_BUFS]
        if t == 0:
            half = (BLK - 1) * NW + (NPIX // 2)
            nc.scalar.activation(tn[:, (BLK - 1) * NW:half],
                                 nraw[:, 0:NPIX // 2], SIGN,
                                 bias=gbias[:, 0:1])
            nc.scalar.activation(tn[:, half:(NH + BLK - 1) * NW],
                                 nraw[:, NPIX // 2:NPIX], SIGN,
                                 bias=gbias[:, 0:1])
        else:
            nc.scalar.activation(trows, nraw[:], SIGN, bias=gbias[:, 0:1])

    # --------------- window-min chain, DVE/Pool split ---------------
    # vertical log-step on tb (pads preset): a = win2, bt = win4,
    # vp = win5 (into padded cols 4..55); horizontal mirror on vp.
    def chain(t):
        tb = tbs[t % 2]
        vp, a2, b2 = (bufs["vp"][t % 2], bufs["a2"][t % 2],
                      bufs["b2"][t % 3])
        vp3 = vp[:].rearrange("p (h w) -> p h w", w=VPW)
        a23 = a2[:].rearrange("p (h w) -> p h w", w=VPW)
        b23 = b2[:].rearrange("p (h w) -> p h w", w=VPW)
        dm3 = dms(t).rearrange("p (h w) -> p h w", w=W)

        bt3 = bt[:].rearrange("p (h w) -> p h w", w=NW)
        tb3 = tb[:].rearrange("p (h w) -> p h w", w=NW)
        if t == 0:
            # tile 0's vertical chain in lo/hi halves chasing the two
            # halves of Sign_0 (the second noise-DMA half lands ~2us
            # after the first): lo needs only tb rows <= 29
            nc.vector.tensor_tensor(
                a[:, 3 * NW:29 * NW], tb[:, 3 * NW:29 * NW],
                tb[:, 4 * NW:30 * NW], MIN)
            nc.vector.tensor_tensor(
                bt[:, NW:27 * NW], a[:, NW:27 * NW],
                a[:, 3 * NW:29 * NW], MIN)
            nc.vector.tensor_tensor(
                vp3[:, 0:23, BLK - 1:BLK - 1 + NW], bt3[:, 0:23, :],
                tb3[:, BLK - 1:23 + BLK - 1, :], MIN)
            nc.vector.tensor_tensor(
                a[:, 29 * NW:56 * NW], tb[:, 29 * NW:56 * NW],
                tb[:, 30 * NW:57 * NW], MIN)
            nc.vector.tensor_tensor(
                bt[:, 27 * NW:56 * NW], a[:, 27 * NW:56 * NW],
                a[:, 29 * NW:58 * NW], MIN)
            nc.vector.tensor_tensor(
                vp3[:, 23:H, BLK - 1:BLK - 1 + NW], bt3[:, 23:H, :],
                tb3[:, 23 + BLK - 1:TROWS, :], MIN)
        else:
            nc.vector.tensor_tensor(
                a[:, 3 * NW:56 * NW], tb[:, 3 * NW:56 * NW],
                tb[:, 4 * NW:57 * NW], MIN)
            nc.vector.tensor_tensor(
                bt[:, NW:56 * NW], a[:, NW:56 * NW],
                a[:, 3 * NW:58 * NW], MIN)
            nc.vector.tensor_tensor(
                vp3[:, :, BLK - 1:BLK - 1 + NW], bt3[:, 0:H, :],
                tb3[:, BLK - 1:TROWS, :], MIN)
        nc.vector.tensor_tensor(
            a23[:, :, 3:56], vp3[:, :, 3:56], vp3[:, :, 4:57], MIN)
        # NOTE: neuronxcc only accepts TensorTensor on DVE — the GpSimd
        # (Pool) engine is limited to tensor_scalar/memset/DMA on the
        # real ISA, so the whole min chain lives on DVE
        if t == NTILES - 1:
            # last tile's horizontal tail in row-halves: each half's
            # mask-relu (the count the collective waits on) can start
            # ~1.7us earlier than a whole-tile op would allow
            hh = H // 2
            for r0, r1 in ((0, hh), (hh, H)):
                nc.vector.tensor_tensor(
                    b23[:, r0:r1, 1:W], a23[:, r0:r1, 1:W],
                    a23[:, r0:r1, 3:58], MIN)
                nc.vector.tensor_tensor(
                    dm3[:, r0:r1, :], b23[:, r0:r1, 0:W],
                    vp3[:, r0:r1, BLK - 1:VPW], MIN)
        else:
            nc.vector.tensor_tensor(
                b23[:, :, 1:W], a23[:, :, 1:W], a23[:, :, 3:58], MIN)
            nc.vector.tensor_tensor(
                dm3[:, :, :], b23[:, :, 0:W], vp3[:, :, BLK - 1:VPW], MIN)

    def mask_dve(t):
        # explicit 2nd op: the ISA's TensorScalarPtrReduce (accum_out)
        # requires both ALU stages
        nc.vector.tensor_scalar(dms(t), dms(t), 0.0, 1.0, IS_GT, MULT,
                                accum_out=partials[:, t:t + 1])

    # smark: a [P,1] zero written via a read of Sign_7's output — every
    # ACT mask-relu takes it as (numerically inert) bias, which pins the
    # relus BEHIND all eight Signs in the compiled ACT order.  Without
    # this the list scheduler interleaves relus between the Signs, and
    # each inversion stalls the noise ring by ~3us.
    smark = stats_pool.tile([P, 1], F32, name=f"smark{rep}", tag="smark")

    def mask_relu(t):
        nc.scalar.activation(dms(t), dms(t), RELU, bias=smark[:, 0:1],
                             accum_out=partials[:, t:t + 1])

    # noise loads + signs just-in-time; chain(t) before sign(t+2) so the
    # tb ring WAR resolves correctly.  The ACT mask-relus are emitted only
    # AFTER the last sign so the compiled ACT order keeps every Sign (which
    # gates the noise ring and the chains) ahead of them.
    load_noise(0)
    sign(0)
    load_noise(1)
    sign(1)
    for t in range(NTILES - 2):
        chain(t)
        if t in DVE_CNT:
            mask_dve(t)
        if t == X_CHUNKS[0][2]:
            load_x_chunk(*X_CHUNKS[0])
        elif t == X_CHUNKS[1][2]:
            load_x_chunk(*X_CHUNKS[1])
        load_noise(t + 2)
        sign(t + 2)
    # smark = (a data element of Sign_7's tb slot) * 0.0
    nc.vector.tensor_scalar(
        smark[:], tbs[(NTILES - 1) % 2][:, (BLK - 1) * NW:(BLK - 1) * NW + 1],
        0.0, None, MULT)
    for t in range(NTILES - 2):
        mask_relu(t)
    chain(NTILES - 2)
    mask_relu(NTILES - 2)
    chain(NTILES - 1)
    # tile 7's mask-relu in halves (matching the split chain tail) so
    # the collective-critical count completes as early as possible
    t7 = (NTILES - 1) * OPIX
    hp = (H // 2) * W
    nc.scalar.activation(dmstore[:, t7:t7 + hp],
                         dmstore[:, t7:t7 + hp], RELU,
                         bias=smark[:, 0:1],
                         accum_out=partials[:, NTILES - 1:NTILES])
    nc.scalar.activation(dmstore[:, t7 + hp:t7 + OPIX],
                         dmstore[:, t7 + hp:t7 + OPIX], RELU,
                         bias=smark[:, 0:1],
                         accum_out=partials[:, NTILES:NTILES + 1])

    # ------------- global count -> scale = M / count_ones -------------
    ptot = stats_pool.tile([P, 1], F32, name=f"ptot{rep}", tag="ptot")
    nc.vector.tensor_reduce(ptot[:], partials[:, 0:NTILES + 1],
                            mybir.AxisListType.X, ADD)
    # cross-partition reduce + broadcast in one idle-PE matmul:
    # psum[m, 0] = sum_p ones[p, m] * ptot[p, 0] = total, for every m.
    ptot_ps = psum_pool.tile([P, 1], F32, name=f"ptot_ps{rep}", tag="pps")
    nc.tensor.matmul(ptot_ps[:], ones_sb[:], ptot[:], start=True, stop=True)
    pall = stats_pool.tile([P, 1], F32, name=f"pall{rep}", tag="pall")
    nc.vector.tensor_copy(pall[:], ptot_ps[:])
    if single_core:
        tot_sb = pall
    else:
        # cc transfers ride the scalar HWDGE queue: the sync queue's FIFO
        # may still hold store traffic
        cc_in = dram_pool.tile([P, 1], F32, name=f"cc_in{rep}", tag="cc_in")
        cc_out = dram_pool.tile([P, 1], F32, name=f"cc_out{rep}",
                                tag="cc_out")
        nc.scalar.dma_start(cc_in[:], pall[:])
        nc.gpsimd.collective_compute(
            "AllReduce", ADD,
            replica_groups=[list(range(N_CORES))],
            ins=[cc_in.opt()], outs=[cc_out.opt()])
        tot_sb = stats_pool.tile([P, 1], F32, name=f"tot{rep}", tag="tot")
        nc.scalar.dma_start(tot_sb[:], cc_out[:])
    recip = stats_pool.tile([P, 1], F32, name=f"recip{rep}", tag="recip")
    nc.vector.reciprocal(recip[:], tot_sb[:])
    scale_sb = stats_pool.tile([P, 1], F32, name=f"scale{rep}", tag="scale")
    nc.vector.tensor_scalar_mul(scale_sb[:], recip[:], COUNT_M)

    # ------- scale-gated tail: out_t = mask_t * xb_t * scale -------
    # every tail op reads scale_sb (directly or transitively), so the
    # static scheduler cannot move any of it into the chain window
    def tail(t):
        if t in POOL_TAIL:
            # msc = mask*scale on Pool (tensor_scalar — GpSimd-legal)
            nc.gpsimd.tensor_scalar(dms(t), dms(t), scale_sb[:, 0:1],
                                    None, MULT)
            nc.vector.tensor_tensor(dms(t), dms(t), xbs(t), MULT)
        elif t in ACT_TAIL:
            # xs = xb*scale on ACT (per-partition activation scale)
            nc.scalar.activation(xbs(t), xbs(t), IDENT,
                                 scale=scale_sb[:, 0:1])
            nc.vector.tensor_tensor(dms(t), dms(t), xbs(t), MULT)
        else:
            nc.vector.tensor_scalar(dms(t), dms(t), scale_sb[:, 0:1],
                                    None, MULT)
            nc.vector.tensor_tensor(dms(t), dms(t), xbs(t), MULT)
        # alternate stores across both HWDGE queues so the final drain is
        # paced by aggregate DMA BW, not one queue's serialization
        eng = nc.sync if t % 2 == 0 else nc.scalar
        eng.dma_start(out_ap[bass.ts(t, P), :], dms(t))

    def tail0_pieces():
        # tile 0's tail in quarter/quarter/half pieces so the store
        # stream (the final DMA-bound 17.8us) starts ~4us earlier
        q = OPIX // 4
        for lo, hi in ((0, q), (q, 2 * q), (2 * q, OPIX)):
            d = dmstore[:, lo:hi]
            nc.vector.tensor_scalar(d, d, scale_sb[:, 0:1], None, MULT)
            nc.vector.tensor_tensor(d, d, xbstore[:, lo:hi], MULT)
            nc.sync.dma_start(out_ap[bass.ts(0, P), lo:hi], d)

    # DVE-local tiles first: their stores flow immediately after scale
    # while the Pool/ACT scale-multiplies for the other tiles cook
    tail0_pieces()
    for t in (2, 5, 1, 3, 4, 6, 7):
        tail(t)


def _get_nc():
    if "nc" not in _CACHE:
        _CACHE["nc"] = _build()
    return _CACHE["nc"]


def kernel(x: np.ndarray, noise: np.ndarray) -> np.ndarray:
    x = np.asarray(x, dtype=np.float32)
    noise = np.asarray(noise, dtype=np.float32)
    assert x.shape == (B, C, H, W) and noise.shape == (B, C, NH, NW)
    nc = _get_nc()
    in_maps = []
    for i in range(N_CORES):
        xs = np.ascontiguousarray(x[i * B_SH:(i + 1) * B_SH]).reshape(
            IMGS, OPIX)
        ns = np.ascontiguousarray(noise[i * B_SH:(i + 1) * B_SH]).reshape(
            IMGS, NPIX)
        in_maps.append({"x": xs, "noise": ns})
    res = run_bass_kernel_spmd(nc, in_maps, list(range(N_CORES)))
    out = np.empty((B, C, H, W), dtype=np.float32)
    for i in range(N_CORES):
        out[i * B_SH:(i + 1) * B_SH] = np.asarray(
            res.results[i]["out"]).astype(np.float32).reshape(
                B_SH, C, H, W)
    return out


# revision 53
# speedup vs baseline: 1.0143x; 1.0000x over previous
"""DropBlock kernel for Trainium2, 8 NeuronCores, batch-sharded data parallel.

Reference computation (B,C,H,W = 128,64,56,56, block=5, gamma=0.02):
    mask    = (noise < gamma)                       # (B,C,52,52) corner drops
    dilated = maxpool5x5_full_pad(mask)             # (B,C,56,56)
    block_mask = 1 - dilated
    out = block_mask * x * (numel / sum(block_mask))

Schedule:
  - The eight noise tiles stream first on the SP HWDGE queue (f32 — the
    gamma compare must be f32-exact).  ACT runs only Sign ops during this
    window (the mask-relus are pinned behind all Signs via an inert bias
    read of a marker written from Sign_7's output), so the 3-slot noise
    ring never throttles the stream.
  - x is loaded by TWO gpsimd (SWDGE) cast-DMAs, f32 DRAM -> bf16 SBUF,
    4 tiles each (DRAM-side AP exposes the tile dim so row t*P+p lands
    on partition p): the DMA bus cost is charged at the bf16 size,
    halving the x read time, and no ACT cast phase exists at all.  Each
    chunk is release-gated by a 1-element DVE tensor_copy reading a
    mid-pipeline chain scratch, so the x transfers queue up behind the
    noise stream instead of racing it for DMA bandwidth.
  - The 5x5 window-min chain (log-step, 6 tensor_tensor ops/tile on +-1
    signs) runs entirely on DVE (2x bf16): neuronxcc rejects
    TensorTensor/ScalarTensorTensor on GpSimd, and ACT/PE have no
    elementwise two-tensor path, so DVE throughput is the kernel's
    floor.  Tile 0's vertical chain is split in halves chasing the two
    Sign_0 halves; tile 7's horizontal tail and mask-relu are split in
    row-halves so the collective-critical count lands early.
  - Masks+counts: ACT Relu in place doubles as the +-1 -> {0,1}
    conversion and the exact per-tile count (accum_out).  (The DVE
    tensor_scalar accumulator writes zeros through neuronxcc/birsim —
    counts must stay on ACT.)
  - Tail per tile (scale-gated, so the static scheduler cannot hoist it
    into the chain window): out = mask*scale (DVE 4x tensor_scalar, or
    Pool tensor_scalar / ACT per-partition-scale activation for three
    tiles each, keeping the multiplies off saturated DVE) then
    out *= xb (DVE 2x).  Tile 0 is processed in quarter/half pieces so
    the DMA-bound store stream starts early; stores alternate across
    the SP and ACT HWDGE queues.
  - scale = COUNT_M / count_global; cross-partition reduce+broadcast is
    one idle-PE matmul against ones; cross-core is a [P,1] AllReduce on
    the ACT HWDGE queue (the SP queue still has x/store traffic).

Exactness: q = Sign(noise - gamma_lo) with the subtract riding the ACT
bias (f32 inside the datapath); q=0 at noise==gamma_lo propagates through
min into mask=0, matching the reference drop.  Counts are integer-exact
in f32; the only roundings are the bf16 cast of x and the bf16 store.
"""

import sys

sys.path.insert(0, "/opt/trn_rl_repo")

import numpy as np

import concourse.bacc as bacc
import concourse.bass as bass
import concourse.tile as tile
import concourse.mybir as mybir
from concourse.bass_utils import run_bass_kernel_spmd

N_CORES = 8
B, C, H, W = 128, 64, 56, 56
BLK = 5
GAMMA = 0.02
NH, NW = H - (BLK - 1), W - (BLK - 1)  # 52, 52 noise dims
B_SH = B // N_CORES  # 16 batches per core
IMGS = B_SH * C  # 1024 images per core
P = 128  # partitions
NTILES = IMGS // P  # 8 tiles per core
NPIX = NH * NW  # 2704 noise pixels/image
OPIX = H * W  # 3136 out pixels/image
TROWS = NH + 2 * (BLK - 1)  # 60 rows in padded vertical buffer
TFLAT = TROWS * NW  # 3120
VPW = NW + 2 * (BLK - 1)  # 60 cols in padded horizontal buffer (4+52+4)
COUNT_M = float(B * C * H * W)  # 25690112.0

# Largest f32 strictly below 0.02f: noise > gamma_lo  <=>  noise >= 0.02f.
GAMMA_LO = float(np.nextafter(np.float32(GAMMA), np.float32(0)))

F32 = mybir.dt.float32
BF16 = mybir.dt.bfloat16
MIN = mybir.AluOpType.min
MULT = mybir.AluOpType.mult
ADD = mybir.AluOpType.add
IS_GT = mybir.AluOpType.is_gt
IDENT = mybir.ActivationFunctionType.Identity
SIGN = mybir.ActivationFunctionType.Sign
RELU = mybir.ActivationFunctionType.Relu

N_BUFS = 3  # noise f32 staging ring
# ALL masks+counts ride ACT Relu with accum_out: the DVE tensor_scalar
# accumulator silently writes zeros through neuronxcc/birsim, so no
# counts may live on DVE
DVE_CNT = ()
# tail engine split: Pool tiles get msc = mask*scale via tensor_scalar
# (the only elementwise op neuronxcc allows on GpSimd), ACT tiles get
# xs = xb*scale via activation with a per-partition scale — both keep
# the scale-multiplies off DVE, which the chain already saturates
POOL_TAIL = (1, 3)
ACT_TAIL = (4, 6, 7)
# x cast-load chunks: (first_tile, n_tiles, gate_tile) — the transfer is
# released by a tiny DVE copy that reads gate_tile's a2 chain scratch
# (DVE-local, so the gate never makes DVE wait on another engine)
X_CHUNKS = ((0, 4, 2), (4, 4, 4))

_CACHE = {}


def _build(single_core=False, repeat=1, no_cc=False):
    """Build + compile the SPMD bass module once.

    single_core=True builds a collective-free variant (the per-core count is
    used directly as the global count) for cost-model simulation only.
    repeat>1 unrolls the whole pipeline k times (benchmarking only).
    no_cc=True skips the AllReduce on the 8-core build (timing probe only —
    results are wrong by the per-core/global count ratio).
    """
    nc = bacc.Bacc("TRN2", target_bir_lowering=False, debug=False,
                   num_devices=1 if single_core else N_CORES)
    noise_ap = nc.dram_tensor("noise", [IMGS, NPIX], F32,
                              kind="ExternalInput").ap()
    x_ap = nc.dram_tensor("x", [IMGS, OPIX], F32, kind="ExternalInput").ap()
    out_ap = nc.dram_tensor("out", [IMGS, OPIX], BF16,
                            kind="ExternalOutput").ap()

    with tile.TileContext(nc) as tc:
        with (
            tc.tile_pool(name="buf", bufs=1) as bp,
            tc.tile_pool(name="stats", bufs=1) as stats_pool,
            tc.tile_pool(name="ps", bufs=1, space="PSUM") as psum_pool,
            tc.tile_pool(name="dram", bufs=1, space="DRAM") as dram_pool,
        ):
            bufs = {
                "nraw": [bp.tile([P, NPIX], F32, name=f"nraw{s}")
                         for s in range(N_BUFS)],
                "tb": [bp.tile([P, TFLAT], BF16, name=f"tb{s}")
                       for s in range(2)],
                "a": [bp.tile([P, (TROWS - 1) * NW], BF16, name="a0")],
                "bt": [bp.tile([P, (TROWS - 3) * NW], BF16, name="bt0")],
                "vp": [bp.tile([P, H * VPW], BF16, name=f"vp{s}")
                       for s in range(2)],
                "a2": [bp.tile([P, H * VPW], BF16, name=f"a2_{s}")
                       for s in range(2)],
                "b2": [bp.tile([P, H * VPW], BF16, name=f"b2_{s}")
                       for s in range(3)],
            }
            dmstore = bp.tile([P, NTILES * OPIX], BF16, name="dmstore")
            xbstore = bp.tile([P, NTILES * OPIX], BF16, name="xbstore")
            ones_sb = bp.tile([P, P], F32, name="ones")
            nc.gpsimd.memset(ones_sb[:], 1.0)
            # one extra slot: tile 7's count arrives as two half-relus
            partials = stats_pool.tile([P, NTILES + 1], F32)
            gbias = stats_pool.tile([P, 1], F32)
            nc.vector.memset(gbias[:], -GAMMA_LO)
            # warm the ACT function tables on a 1-element tile during the
            # DMA lead-in; otherwise LoadActFuncSet (~1.3us) lands in front
            # of the first real op and stalls the pipeline.
            warm = stats_pool.tile([P, 1], F32)
            nc.scalar.activation(warm[:], gbias[:], IDENT, bias=gbias[:, 0:1])
            nc.scalar.activation(warm[:], warm[:], SIGN)
            nc.scalar.activation(warm[:], warm[:], RELU)

            # 1.0-pads written once; every later op touches only data rows.
            for tb in bufs["tb"]:
                nc.gpsimd.memset(tb[:, 0:(BLK - 1) * NW], 1.0)
                nc.gpsimd.memset(tb[:, (NH + BLK - 1) * NW:TFLAT], 1.0)
            a0 = bufs["a"][0]
            nc.gpsimd.memset(a0[:, 0:3 * NW], 1.0)
            nc.gpsimd.memset(a0[:, 56 * NW:(TROWS - 1) * NW], 1.0)
            bt0 = bufs["bt"][0]
            nc.gpsimd.memset(bt0[:, 0:NW], 1.0)
            nc.gpsimd.memset(bt0[:, 56 * NW:(TROWS - 3) * NW], 1.0)
            for vp in bufs["vp"]:
                vp3 = vp[:].rearrange("p (h w) -> p h w", w=VPW)
                nc.gpsimd.memset(vp3[:, :, 0:BLK - 1], 1.0)
                nc.gpsimd.memset(vp3[:, :, W:VPW], 1.0)
            # horizontal-chain columns that only ever see vp pads are
            # constant 1.0: a2 cols 0..2 and 56..57, b2 col 0.
            for a2 in bufs["a2"]:
                a23i = a2[:].rearrange("p (h w) -> p h w", w=VPW)
                nc.gpsimd.memset(a23i[:, :, 0:3], 1.0)
                nc.gpsimd.memset(a23i[:, :, 56:58], 1.0)
            for b2 in bufs["b2"]:
                b23i = b2[:].rearrange("p (h w) -> p h w", w=VPW)
                nc.gpsimd.memset(b23i[:, :, 0:1], 1.0)

            for rep in range(repeat):
                _emit_once(nc, tc, noise_ap, x_ap, out_ap, bufs, dmstore,
                           xbstore, ones_sb, partials, gbias, stats_pool,
                           psum_pool, dram_pool, single_core or no_cc, rep)

    nc.compile()
    return nc


def _emit_once(nc, tc, noise_ap, x_ap, out_ap, bufs, dmstore, xbstore,
               ones_sb, partials, gbias, stats_pool, psum_pool, dram_pool,
               single_core, rep):
    nraws, tbs = bufs["nraw"], bufs["tb"]
    a, bt = bufs["a"][0], bufs["bt"][0]

    def dms(t):
        return dmstore[:, t * OPIX:(t + 1) * OPIX]

    def xbs(t):
        return xbstore[:, t * OPIX:(t + 1) * OPIX]

    def load_noise(t):
        nraw = nraws[t % N_BUFS]
        if t == 0:
            # halve the cold-start DMA latency: Sign_0 on the first half
            # can begin as soon as rows 0..25 have landed
            nc.sync.dma_start(nraw[:, 0:NPIX // 2],
                              noise_ap[bass.ts(t, P), 0:NPIX // 2])
            nc.sync.dma_start(nraw[:, NPIX // 2:NPIX],
                              noise_ap[bass.ts(t, P), NPIX // 2:NPIX])
        else:
            nc.sync.dma_start(nraw[:], noise_ap[bass.ts(t, P), :])

    def load_x_chunk(t0, n, gate_t):
        # release gate: a 1-element DVE copy that reads tile gate_t's a2
        # scratch — the cast-DMA (WAW on xbstore) cannot start, and
        # therefore cannot steal DMA bandwidth from the noise stream,
        # before the chain pipeline has reached gate_t.  a2 is DVE-local,
        # so the gate costs DVE nothing.
        # read a DATA column of a2 (col 3) — cols 0..2 are pad constants
        # written once at setup, which would release the gate immediately
        nc.vector.tensor_copy(xbstore[:, t0 * OPIX:t0 * OPIX + 1],
                              bufs["a2"][gate_t % 2][:, 3:4])
        # DRAM row t*P+p must land on partition p, tile-block t: expose
        # the tile dim as a middle AP dim on both sides
        src = x_ap[t0 * P:(t0 + n) * P, :].rearrange(
            "(t p) c -> p t c", p=P)
        dst = xbstore[:, t0 * OPIX:(t0 + n) * OPIX].rearrange(
            "p (t c) -> p t c", c=OPIX)
        nc.gpsimd.dma_start(dst, src)

    def sign(t):
        tn = tbs[t % 2]
        trows = tn[:, (BLK - 1) * NW:(NH + BLK - 1) * NW]
        nraw = nraws[t % N_BUFS]
        if t == 0:
            half = (BLK - 1) * NW + (NPIX // 2)
            nc.scalar.activation(tn[:, (BLK - 1) * NW:half],
                                 nraw[:, 0:NPIX // 2], SIGN,
                                 bias=gbias[:, 0:1])
            nc.scalar.activation(tn[:, half:(NH + BLK - 1) * NW],
                                 nraw[:, NPIX // 2:NPIX], SIGN,
                                 bias=gbias[:, 0:1])
        else:
            nc.scalar.activation(trows, nraw[:], SIGN, bias=gbias[:, 0:1])

    # --------------- window-min chain, DVE/Pool split ---------------
    # vertical log-step on tb (pads preset): a = win2, bt = win4,
    # vp = win5 (into padded cols 4..55); horizontal mirror on vp.
    def chain(t):
        tb = tbs[t % 2]
        vp, a2, b2 = (bufs["vp"][t % 2], bufs["a2"][t % 2],
                      bufs["b2"][t % 3])
        vp3 = vp[:].rearrange("p (h w) -> p h w", w=VPW)
        a23 = a2[:].rearrange("p (h w) -> p h w", w=VPW)
        b23 = b2[:].rearrange("p (h w) -> p h w", w=VPW)
        dm3 = dms(t).rearrange("p (h w) -> p h w", w=W)

        bt3 = bt[:].rearrange("p (h w) -> p h w", w=NW)
        tb3 = tb[:].rearrange("p (h w) -> p h w", w=NW)
        if t == 0:
            # tile 0's vertical chain in lo/hi halves chasing the two
            # halves of Sign_0 (the second noise-DMA half lands ~2us
            # after the first): lo needs only tb rows <= 29
            nc.vector.tensor_tensor(
                a[:, 3 * NW:29 * NW], tb[:, 3 * NW:29 * NW],
                tb[:, 4 * NW:30 * NW], MIN)
            nc.vector.tensor_tensor(
                bt[:, NW:27 * NW], a[:, NW:27 * NW],
                a[:, 3 * NW:29 * NW], MIN)
            nc.vector.tensor_tensor(
                vp3[:, 0:23, BLK - 1:BLK - 1 + NW], bt3[:, 0:23, :],
                tb3[:, BLK - 1:23 + BLK - 1, :], MIN)
            nc.vector.tensor_tensor(
                a[:, 29 * NW:56 * NW], tb[:, 29 * NW:56 * NW],
                tb[:, 30 * NW:57 * NW], MIN)
            nc.vector.tensor_tensor(
                bt[:, 27 * NW:56 * NW], a[:, 27 * NW:56 * NW],
                a[:, 29 * NW:58 * NW], MIN)
            nc.vector.tensor_tensor(
                vp3[:, 23:H, BLK - 1:BLK - 1 + NW], bt3[:, 23:H, :],
                tb3[:, 23 + BLK - 1:TROWS, :], MIN)
        else:
            nc.vector.tensor_tensor(
                a[:, 3 * NW:56 * NW], tb[:, 3 * NW:56 * NW],
                tb[:, 4 * NW:57 * NW], MIN)
            nc.vector.tensor_tensor(
                bt[:, NW:56 * NW], a[:, NW:56 * NW],
                a[:, 3 * NW:58 * NW], MIN)
            nc.vector.tensor_tensor(
                vp3[:, :, BLK - 1:BLK - 1 + NW], bt3[:, 0:H, :],
                tb3[:, BLK - 1:TROWS, :], MIN)
        nc.vector.tensor_tensor(
            a23[:, :, 3:56], vp3[:, :, 3:56], vp3[:, :, 4:57], MIN)
        # NOTE: neuronxcc only accepts TensorTensor on DVE — the GpSimd
        # (Pool) engine is limited to tensor_scalar/memset/DMA on the
        # real ISA, so the whole min chain lives on DVE
        if t == NTILES - 1:
            # last tile's horizontal tail in asymmetric row-pieces: the
            # big piece's mask-relu runs while the small piece computes,
            # and the FINAL relu (the op the collective waits on) is only
            # a quarter-tile
            hh = 42
            for r0, r1 in ((0, hh), (hh, H)):
                nc.vector.tensor_tensor(
                    b23[:, r0:r1, 1:W], a23[:, r0:r1, 1:W],
                    a23[:, r0:r1, 3:58], MIN)
                nc.vector.tensor_tensor(
                    dm3[:, r0:r1, :], b23[:, r0:r1, 0:W],
                    vp3[:, r0:r1, BLK - 1:VPW], MIN)
        else:
            nc.vector.tensor_tensor(
                b23[:, :, 1:W], a23[:, :, 1:W], a23[:, :, 3:58], MIN)
            nc.vector.tensor_tensor(
                dm3[:, :, :], b23[:, :, 0:W], vp3[:, :, BLK - 1:VPW], MIN)

    def mask_dve(t):
        # explicit 2nd op: the ISA's TensorScalarPtrReduce (accum_out)
        # requires both ALU stages
        nc.vector.tensor_scalar(dms(t), dms(t), 0.0, 1.0, IS_GT, MULT,
                                accum_out=partials[:, t:t + 1])

    # smark: a [P,1] zero written via a read of Sign_7's output — every
    # ACT mask-relu takes it as (numerically inert) bias, which pins the
    # relus BEHIND all eight Signs in the compiled ACT order.  Without
    # this the list scheduler interleaves relus between the Signs, and
    # each inversion stalls the noise ring by ~3us.
    smark = stats_pool.tile([P, 1], F32, name=f"smark{rep}", tag="smark")

    def mask_relu(t):
        nc.scalar.activation(dms(t), dms(t), RELU, bias=smark[:, 0:1],
                             accum_out=partials[:, t:t + 1])

    # noise loads + signs just-in-time; chain(t) before sign(t+2) so the
    # tb ring WAR resolves correctly.  The ACT mask-relus are emitted only
    # AFTER the last sign so the compiled ACT order keeps every Sign (which
    # gates the noise ring and the chains) ahead of them.
    load_noise(0)
    sign(0)
    load_noise(1)
    sign(1)
    for t in range(NTILES - 2):
        chain(t)
        if t in DVE_CNT:
            mask_dve(t)
        if t == X_CHUNKS[0][2]:
            load_x_chunk(*X_CHUNKS[0])
        elif t == X_CHUNKS[1][2]:
            load_x_chunk(*X_CHUNKS[1])
        load_noise(t + 2)
        sign(t + 2)
    # smark = (a data element of Sign_7's tb slot) * 0.0
    nc.vector.tensor_scalar(
        smark[:], tbs[(NTILES - 1) % 2][:, (BLK - 1) * NW:(BLK - 1) * NW + 1],
        0.0, None, MULT)
    for t in range(NTILES - 2):
        mask_relu(t)
    chain(NTILES - 2)
    mask_relu(NTILES - 2)
    chain(NTILES - 1)
    # tile 7's mask-relu in halves (matching the split chain tail) so
    # the collective-critical count completes as early as possible
    t7 = (NTILES - 1) * OPIX
    hp = 42 * W
    nc.scalar.activation(dmstore[:, t7:t7 + hp],
                         dmstore[:, t7:t7 + hp], RELU,
                         bias=smark[:, 0:1],
                         accum_out=partials[:, NTILES - 1:NTILES])
    nc.scalar.activation(dmstore[:, t7 + hp:t7 + OPIX],
                         dmstore[:, t7 + hp:t7 + OPIX], RELU,
                         bias=smark[:, 0:1],
                         accum_out=partials[:, NTILES:NTILES + 1])

    # ------------- global count -> scale = M / count_ones -------------
    ptot = stats_pool.tile([P, 1], F32, name=f"ptot{rep}", tag="ptot")
    nc.vector.tensor_reduce(ptot[:], partials[:, 0:NTILES + 1],
                            mybir.AxisListType.X, ADD)
    # cross-partition reduce + broadcast in one idle-PE matmul:
    # psum[m, 0] = sum_p ones[p, m] * ptot[p, 0] = total, for every m.
    ptot_ps = psum_pool.tile([P, 1], F32, name=f"ptot_ps{rep}", tag="pps")
    nc.tensor.matmul(ptot_ps[:], ones_sb[:], ptot[:], start=True, stop=True)
    pall = stats_pool.tile([P, 1], F32, name=f"pall{rep}", tag="pall")
    nc.vector.tensor_copy(pall[:], ptot_ps[:])
    if single_core:
        tot_sb = pall
    else:
        # cc transfers ride the scalar HWDGE queue: the sync queue's FIFO
        # may still hold store traffic
        cc_in = dram_pool.tile([P, 1], F32, name=f"cc_in{rep}", tag="cc_in")
        cc_out = dram_pool.tile([P, 1], F32, name=f"cc_out{rep}",
                                tag="cc_out")
        nc.scalar.dma_start(cc_in[:], pall[:])
        nc.gpsimd.collective_compute(
            "AllReduce", ADD,
            replica_groups=[list(range(N_CORES))],
            ins=[cc_in.opt()], outs=[cc_out.opt()])
        tot_sb = stats_pool.tile([P, 1], F32, name=f"tot{rep}", tag="tot")
        nc.scalar.dma_start(tot_sb[:], cc_out[:])
    recip = stats_pool.tile([P, 1], F32, name=f"recip{rep}", tag="recip")
    nc.vector.reciprocal(recip[:], tot_sb[:])
    scale_sb = stats_pool.tile([P, 1], F32, name=f"scale{rep}", tag="scale")
    nc.vector.tensor_scalar_mul(scale_sb[:], recip[:], COUNT_M)

    # ------- scale-gated tail: out_t = mask_t * xb_t * scale -------
    # every tail op reads scale_sb (directly or transitively), so the
    # static scheduler cannot move any of it into the chain window
    def tail(t):
        if t in POOL_TAIL:
            # msc = mask*scale on Pool (tensor_scalar — GpSimd-legal)
            nc.gpsimd.tensor_scalar(dms(t), dms(t), scale_sb[:, 0:1],
                                    None, MULT)
            nc.vector.tensor_tensor(dms(t), dms(t), xbs(t), MULT)
        elif t in ACT_TAIL:
            # xs = xb*scale on ACT (per-partition activation scale)
            nc.scalar.activation(xbs(t), xbs(t), IDENT,
                                 scale=scale_sb[:, 0:1])
            nc.vector.tensor_tensor(dms(t), dms(t), xbs(t), MULT)
        else:
            nc.vector.tensor_scalar(dms(t), dms(t), scale_sb[:, 0:1],
                                    None, MULT)
            nc.vector.tensor_tensor(dms(t), dms(t), xbs(t), MULT)
        # alternate stores across both HWDGE queues so the final drain is
        # paced by aggregate DMA BW, not one queue's serialization
        eng = nc.sync if t % 2 == 0 else nc.scalar
        eng.dma_start(out_ap[bass.ts(t, P), :], dms(t))

    def tail0_pieces():
        # tile 0's tail in quarter/quarter/half pieces so the store
        # stream (the final DMA-bound 17.8us) starts ~4us earlier
        q = OPIX // 4
        for lo, hi in ((0, q), (q, 2 * q), (2 * q, OPIX)):
            d = dmstore[:, lo:hi]
            nc.vector.tensor_scalar(d, d, scale_sb[:, 0:1], None, MULT)
            nc.vector.tensor_tensor(d, d, xbstore[:, lo:hi], MULT)
            nc.sync.dma_start(out_ap[bass.ts(0, P), lo:hi], d)

    # DVE-local tiles first: their stores flow immediately after scale
    # while the Pool/ACT scale-multiplies for the other tiles cook
    tail0_pieces()
    for t in (2, 5, 1, 3, 4, 6, 7):
        tail(t)


def _get_nc():
    if "nc" not in _CACHE:
        _CACHE["nc"] = _build()
    return _CACHE["nc"]


def kernel(x: np.ndarray, noise: np.ndarray) -> np.ndarray:
    x = np.asarray(x, dtype=np.float32)
    noise = np.asarray(noise, dtype=np.float32)
    assert x.shape == (B, C, H, W) and noise.shape == (B, C, NH, NW)
    nc = _get_nc()
    in_maps = []
    for i in range(N_CORES):
        xs = np.ascontiguousarray(x[i * B_SH:(i + 1) * B_SH]).reshape(
            IMGS, OPIX)
        ns = np.ascontiguousarray(noise[i * B_SH:(i + 1) * B_SH]).reshape(
            IMGS, NPIX)
        in_maps.append({"x": xs, "noise": ns})
    res = run_bass_kernel_spmd(nc, in_maps, list(range(N_CORES)))
    out = np.empty((B, C, H, W), dtype=np.float32)
    for i in range(N_CORES):
        out[i * B_SH:(i + 1) * B_SH] = np.asarray(
            res.results[i]["out"]).astype(np.float32).reshape(
                B_SH, C, H, W)
    return out
